# revision 16
# baseline (speedup 1.0000x reference)
"""Trainium2 Bass kernel for a hierarchical RNN language model (train branch).

Model (B=64, L=32, V=32000, E=512, H=1024):
  emb   = embedding[x]                                  # gather
  sent  = tanh(mean_l(emb sections) @ W_csm)            # [B,3,H]
  hs    = 2-layer tanh RNN over the 3 sentence vectors  # [3,B,H]
  ctx   = tanh(hs @ U[l])            per position l     # [3,B,H]
  cur   = tanh(Ww[word] + ctx @ Wc)  positions l=1..31
  y_sec = cur @ Wfc                                     # [3,B,31,V]  << dominant
  y     = concat(one_hot(first words), y_sec)           # [B,96,V]

Distribution over 8 NeuronCores: the per-position work (ctx/cur/final fc,
which selects U[l]) is sharded by position l: 4 slots per core (core 7
carries one dummy slot).  The tiny CSM+RNN prologue is replicated on all
cores.  Device activations live in a transposed layout (features on SBUF
partitions, (section,batch) on the free axis) so the whole chain runs with
weights as the stationary matmul operand and zero activation transposes.
The dominant matmul runs in fp16 with fp32 PSUM accumulation.

Perf structure (vs the first working version):
  - all tables/weights are fp16 in DRAM (host-cast): halves load bytes
  - the embedding rows for the CSM window and the Ww rows for this core's
    words are staged host-side into dense per-core input tensors (same
    host marshalling as the per-core U[lv] slice), already in the SBUF
    tile layout; on device they are plain streaming DMAs.  The per-row
    indirect-DMA path burned ~1us of gpsimd descriptor generation per
    128 rows, serializing the prologue.  (Multi-row batched indirect
    gathers are NOT used: hardware honors only one offset per partition,
    unlike CoreSim.)
  - DMAs are issued in order of first use so each phase's data arrives
    just ahead of its compute; U tiles live in a dedicated top-level pool
    (a phase-local pool would alias phase-C addresses, and the WAR
    dependency then stalls the U loads until C drains)
  - y is written fp16 (halves the 98MB/core store traffic); host upcasts
"""

import sys

for _p in ("/opt/trn_rl_repo", "/root/.axon_site/_ro/trn_rl_repo"):
    if _p not in sys.path:
        sys.path.append(_p)

import numpy as np

import concourse.bass as bass
import concourse.mybir as mybir
import concourse.tile as tile
from concourse import bacc
from concourse.bass_utils import run_bass_kernel_spmd

# ---- problem constants (hardcoded; kernel.py must be self-contained) ----
B, L, V, E, H = 64, 32, 32000, 512, 1024
S = 3                    # sections per example
G = S * B                # 192 activation columns, col = s*B + b
NCORE = 8
LSLOT = 4                # l-positions handled per core
ROWS = LSLOT * G         # 768 output rows per core, row = ls*G + s*B + b
P = 128
ESUB = E // P            # 4
HSUB = H // P            # 8
VCHUNK = 500             # vocab chunk width (psum bank = 512 fp32 max)
NCHUNK = V // VCHUNK     # 64
EMB_TILES = G * L // P   # 48 token tiles for the CSM embedding sum
ROW_TILES = ROWS // P    # 6
GPT = P // L             # 4 (s,b) groups per 128-token tile
NB = 4                   # embedding stream batches
BT = EMB_TILES // NB     # 12 token tiles per batch

# core j handles positions LMAP[j]; position 0 is the host-side one-hot row.
LMAP = [[4 * j + 1, 4 * j + 2, 4 * j + 3, 4 * j + 4] for j in range(7)]
LMAP.append([29, 30, 31, 31])  # last slot of core 7 is a discarded dummy

F16 = mybir.dt.float16
F32 = mybir.dt.float32
I32 = mybir.dt.int32
TANH = mybir.ActivationFunctionType.Tanh

SKIP_PROLOGUE = False  # timing-only: phase E on dummy activations


def build_module(nv_chunks: int = NCHUNK, reps: int = 1):
    """reps>1 wraps the whole body in a hardware loop — used only by the
    benchmark harness to amortize the host->device dispatch latency."""
    nc = bacc.Bacc(None, target_bir_lowering=False, debug=False)

    # ega[p, t, :] = embedding[x-token of CSM row t*P+p]   (host-staged)
    ega = nc.dram_tensor("ega", [P, EMB_TILES, E], F16, kind="ExternalInput")
    # wwg[p, hb, r] = Ww[word_r][hb*P + p]                 (host-staged ^T)
    wwg = nc.dram_tensor("wwg", [P, HSUB, ROWS], F16, kind="ExternalInput")
    mc = nc.dram_tensor("mc", [P, GPT], F16, kind="ExternalInput")
    w_csm = nc.dram_tensor("w_csm", [E, H], F16, kind="ExternalInput")
    wx1 = nc.dram_tensor("wx1", [H, H], F16, kind="ExternalInput")
    wh1 = nc.dram_tensor("wh1", [H, H], F16, kind="ExternalInput")
    wx2 = nc.dram_tensor("wx2", [H, H], F16, kind="ExternalInput")
    wh2 = nc.dram_tensor("wh2", [H, H], F16, kind="ExternalInput")
    u_sh = nc.dram_tensor("u_sh", [LSLOT, H, H], F16, kind="ExternalInput")
    wc = nc.dram_tensor("wc", [H, H], F16, kind="ExternalInput")
    wfc = nc.dram_tensor("wfc", [H, V], F16, kind="ExternalInput")
    y = nc.dram_tensor("y", [ROWS, V], F16, kind="ExternalOutput")

    def kpart(ap2d, sub):  # [K*P, N] dram -> [P, sub, N] (K on partitions)
        return ap2d.ap().rearrange("(s p) n -> p s n", p=P)

    with tile.TileContext(nc) as tc:
        with (
            tc.tile_pool(name="const", bufs=1) as const,
            tc.tile_pool(name="wpool", bufs=1) as wpool,
            tc.tile_pool(name="upool", bufs=1) as upool,
            tc.tile_pool(name="persist", bufs=1) as persist,
        ):
            mc_sb = const.tile([P, GPT], F16)
            nc.sync.dma_start(mc_sb[:], mc.ap())

            # dense-weight tiles; DMAs below are issued in order of first
            # use so each phase's data arrives just ahead of its compute.
            wcsm_sb = wpool.tile([P, ESUB, H], F16)
            wx1_sb = wpool.tile([P, HSUB, H], F16)
            wh1_sb = wpool.tile([P, HSUB, H], F16)
            wx2_sb = wpool.tile([P, HSUB, H], F16)
            wh2_sb = wpool.tile([P, HSUB, H], F16)
            wc_sb = wpool.tile([P, HSUB, H], F16)
            # U tiles live in their own top-level pool: a phase-local pool
            # would alias phase-C's addresses and the WAR dependency then
            # stalls the U loads (hence phase D) until C fully drains.
            u_sbs = [upool.tile([P, HSUB, H], F16, tag="u", bufs=2,
                                name=f"u{ls}") for ls in range(LSLOT)]

            a_t = persist.tile([P, ESUB, G], F16)      # (1/L-unscaled) emb sums^T
            sent_t = persist.tile([P, HSUB, G], F16)   # sentence vectors^T
            h1_t = persist.tile([P, HSUB, G], F16)     # RNN layer-1 hiddens^T
            hs_t = persist.tile([P, HSUB, G], F16)     # RNN layer-2 hiddens^T
            cur_t = persist.tile([P, HSUB, ROWS], F16)
            wwg_t = persist.tile([P, HSUB, ROWS], F16)  # Ww rows^T

            from contextlib import ExitStack as _ES
            _loop_es = _ES()
            if reps > 1:
                _loop_es.enter_context(tc.For_i(0, reps, 1))
            if SKIP_PROLOGUE:
                nc.gpsimd.memset(cur_t[:], 0.01)

            # ---- Phase A: embedding stream + per-sentence token sum.
            # Streamed tile t holds tokens of groups 4t..4t+3 (32 tokens
            # each); summing within a group is a matmul with the
            # block-ones matrix mc.
            with (
                tc.tile_pool(name="pA", bufs=1) as pA,
                tc.tile_pool(name="psA", bufs=2, space="PSUM") as psA,
            ):
                egs = []
                for h in range(0 if SKIP_PROLOGUE else NB):
                    eg = pA.tile([P, BT, E], F16, tag="eg", bufs=2)
                    nc.sync.dma_start(
                        eg[:], ega.ap()[:, h * BT:(h + 1) * BT, :])
                    egs.append(eg)
                if not SKIP_PROLOGUE:
                    # Ww rows arrive pre-transposed; plain copy into SBUF
                    nc.sync.dma_start(wwg_t[:], wwg.ap())
                # weights for phases B/C/D, behind the activations'
                # streams in queue order, ahead of their consuming phase
                nc.sync.dma_start(wcsm_sb[:], kpart(w_csm, ESUB))
                nc.sync.dma_start(wx1_sb[:], kpart(wx1, HSUB))
                nc.sync.dma_start(wh1_sb[:], kpart(wh1, HSUB))
                nc.sync.dma_start(
                    u_sbs[0][:], u_sh.ap()[0].rearrange("(s p) k -> p s k",
                                                        p=P))
                nc.sync.dma_start(wx2_sb[:], kpart(wx2, HSUB))
                nc.sync.dma_start(wh2_sb[:], kpart(wh2, HSUB))
                nc.sync.dma_start(wc_sb[:], kpart(wc, HSUB))
                nc.sync.dma_start(
                    u_sbs[1][:], u_sh.ap()[1].rearrange("(s p) k -> p s k",
                                                        p=P))
                # u2/u3 share slots with u0/u1: their issue waits for phase
                # D's first slots to drain, so they go last on the queue
                for ls in (2, 3):
                    nc.sync.dma_start(
                        u_sbs[ls][:],
                        u_sh.ap()[ls].rearrange("(s p) k -> p s k", p=P))

                accs = [psA.tile([P, G], F32, name=f"accA{m}", bufs=1)
                        for m in range(ESUB)]
                for h in range(0 if SKIP_PROLOGUE else NB):
                    eg = egs[h]
                    for ht in range(BT):
                        t = h * BT + ht
                        for m in range(ESUB):
                            nc.tensor.matmul(
                                accs[m][:, t * GPT:(t + 1) * GPT],
                                eg[:, ht, m * P:(m + 1) * P], mc_sb[:],
                                start=True, stop=True,
                            )
                for m in range(0 if SKIP_PROLOGUE else ESUB):
                    nc.vector.tensor_copy(out=a_t[:, m, :], in_=accs[m][:])

            # ---- Phase B: sent^T = tanh((1/L) * W_csm^T @ a_t)
            with tc.tile_pool(name="psB", bufs=2, space="PSUM") as psB:
                for m in range(0 if SKIP_PROLOGUE else HSUB):
                    acc = psB.tile([P, G], F32, tag="accB")
                    for k in range(ESUB):
                        nc.tensor.matmul(
                            acc[:], wcsm_sb[:, k, m * P:(m + 1) * P],
                            a_t[:, k, :],
                            start=(k == 0), stop=(k == ESUB - 1),
                        )
                    nc.scalar.activation(sent_t[:, m, :], acc[:], TANH,
                                         scale=1.0 / L)

            # ---- Phase C: 2-layer tanh RNN over the 3 sentence steps
            with (
                tc.tile_pool(name="pC", bufs=1) as pC,
                tc.tile_pool(name="psC", bufs=2, space="PSUM") as psC,
            ):
                def input_proj(wsb, src_t, dst):
                    # dst = w^T @ src for all 3 steps at once (input-side term)
                    for m in range(HSUB):
                        acc = psC.tile([P, G], F32, tag="accCp")
                        for k in range(HSUB):
                            nc.tensor.matmul(
                                acc[:], wsb[:, k, m * P:(m + 1) * P],
                                src_t[:, k, :],
                                start=(k == 0), stop=(k == HSUB - 1),
                            )
                        nc.vector.tensor_copy(out=dst[:, m, :], in_=acc[:])

                def recur(whsb, pin, hout):
                    # hout[:, :, s] = tanh(pin[s] + wh^T @ hout[s-1])
                    for s in range(S):
                        for m in range(HSUB):
                            lo, hi = s * B, (s + 1) * B
                            if s == 0:
                                nc.scalar.activation(
                                    hout[:, m, lo:hi], pin[:, m, lo:hi], TANH)
                                continue
                            acc = psC.tile([P, B], F32, tag="accCr")
                            for k in range(HSUB):
                                nc.tensor.matmul(
                                    acc[:], whsb[:, k, m * P:(m + 1) * P],
                                    hout[:, k, lo - B:hi - B],
                                    start=(k == 0), stop=(k == HSUB - 1),
                                )
                            tmp = pC.tile([P, B], F32, tag="tmpC", bufs=2)
                            nc.vector.tensor_add(tmp[:], acc[:],
                                                 pin[:, m, lo:hi])
                            nc.scalar.activation(hout[:, m, lo:hi], tmp[:],
                                                 TANH)

                if not SKIP_PROLOGUE:
                    p1 = pC.tile([P, HSUB, G], F32)
                    input_proj(wx1_sb, sent_t, p1)
                    recur(wh1_sb, p1, h1_t)
                    p2 = pC.tile([P, HSUB, G], F32)
                    input_proj(wx2_sb, h1_t, p2)
                    recur(wh2_sb, p2, hs_t)

            # ---- Phase D: per position slot: ctx = tanh(U_l^T @ hs),
            #              cur = tanh(Wc^T @ ctx + Ww rows)
            with (
                tc.tile_pool(name="pD", bufs=2) as pD,
                tc.tile_pool(name="psD", bufs=2, space="PSUM") as psD,
            ):
                for ls in range(0 if SKIP_PROLOGUE else LSLOT):
                    u_sb = u_sbs[ls]
                    ctx_t = pD.tile([P, HSUB, G], F16, tag="ctx")
                    for kt in range(HSUB):
                        acc = psD.tile([P, G], F32, tag="accD")
                        for k in range(HSUB):
                            nc.tensor.matmul(
                                acc[:], u_sb[:, k, kt * P:(kt + 1) * P],
                                hs_t[:, k, :],
                                start=(k == 0), stop=(k == HSUB - 1),
                            )
                        nc.scalar.activation(ctx_t[:, kt, :], acc[:], TANH)
                    for m in range(HSUB):
                        acc = psD.tile([P, G], F32, tag="accD2")
                        for k in range(HSUB):
                            nc.tensor.matmul(
                                acc[:], wc_sb[:, k, m * P:(m + 1) * P],
                                ctx_t[:, k, :],
                                start=(k == 0), stop=(k == HSUB - 1),
                            )
                        lo, hi = ls * G, (ls + 1) * G
                        tmp = pD.tile([P, G], F32, tag="tmpD", bufs=2)
                        nc.vector.tensor_add(tmp[:], acc[:],
                                             wwg_t[:, m, lo:hi])
                        nc.scalar.activation(cur_t[:, m, lo:hi], tmp[:], TANH)

            # ---- Phase E: y = cur @ Wfc, streamed over vocab chunks
            with (
                tc.tile_pool(name="pE", bufs=3) as pE,
                tc.tile_pool(name="oE", bufs=4) as oE,
                tc.tile_pool(name="psE", bufs=4, space="PSUM") as psE,
            ):
                wfc_ap = kpart(wfc, HSUB)
                for c in range(nv_chunks):
                    wf = pE.tile([P, HSUB, VCHUNK], F16, tag="wf")
                    nc.sync.dma_start(
                        wf[:], wfc_ap[:, :, c * VCHUNK:(c + 1) * VCHUNK])
                    for rt in range(ROW_TILES):
                        acc = psE.tile([P, VCHUNK], F32, tag="accE")
                        for k in range(HSUB):
                            nc.tensor.matmul(
                                acc[:], cur_t[:, k, rt * P:(rt + 1) * P],
                                wf[:, k, :],
                                start=(k == 0), stop=(k == HSUB - 1),
                            )
                        o = oE.tile([P, VCHUNK], F16, tag="o")
                        nc.vector.tensor_copy(out=o[:], in_=acc[:])
                        nc.sync.dma_start(
                            y.ap()[rt * P:(rt + 1) * P,
                                   c * VCHUNK:(c + 1) * VCHUNK], o[:])

            _loop_es.close()

    nc.compile()
    return nc


_module_cache: dict = {}


def get_module(nv_chunks: int = NCHUNK):
    if nv_chunks not in _module_cache:
        _module_cache[nv_chunks] = build_module(nv_chunks)
    return _module_cache[nv_chunks]


def make_in_maps(x, embedding, W_csm, Wx1, Wh1, Wx2, Wh2, U, Ww, Wc, Wfc):
    """Build the 8 per-core input dicts from the full inputs.

    Index-selects (embedding rows for the CSM window, Ww rows per core,
    U slices per core) happen here, in input marshalling; all model
    arithmetic runs on device.
    """
    x = np.asarray(x, dtype=np.int64)
    f16 = lambda a: np.ascontiguousarray(np.asarray(a), dtype=np.float16)

    emb16 = np.asarray(embedding, dtype=np.float16)
    ww16 = np.asarray(Ww, dtype=np.float16)

    # CSM token order: row r = (s*B + b)*L + lt  ->  token x[b, s*L + lt]
    xi = x[:, :S * L].reshape(B, S, L)                  # [b, s, lt]
    emb_idx = xi.transpose(1, 0, 2).reshape(-1)         # row r order
    # device tile t, partition p holds row t*P + p
    ega = emb16[emb_idx.reshape(EMB_TILES, P)]          # [T, P, E]
    ega = np.ascontiguousarray(ega.transpose(1, 0, 2))  # [P, T, E]

    mc_np = np.zeros((P, GPT), np.float16)
    mc_np[np.arange(P), np.arange(P) // L] = 1.0

    shared = dict(
        ega=ega, mc=mc_np,
        w_csm=f16(W_csm), wx1=f16(Wx1), wh1=f16(Wh1),
        wx2=f16(Wx2), wh2=f16(Wh2), wc=f16(Wc), wfc=f16(Wfc),
    )
    U = np.asarray(U)
    in_maps = []
    for j in range(NCORE):
        lv = np.array(LMAP[j])                          # [LSLOT]
        # word index for (ls, s, b): x[b, (s+1)*L + l - 1]
        cols = (np.arange(S) + 1)[None, :] * L + lv[:, None] - 1   # [LSLOT, S]
        wwi = x[:, cols].transpose(1, 2, 0).reshape(-1)  # row = ls*G + s*B + b
        wrows = ww16[wwi]                                # [ROWS, H]
        # wwg[p, hb, r] = wrows[r, hb*P + p]
        wwg = np.ascontiguousarray(
            wrows.reshape(ROWS, HSUB, P).transpose(2, 1, 0))
        m = dict(shared)
        m["u_sh"] = f16(U[lv])
        m["wwg"] = wwg
        in_maps.append(m)
    return in_maps


def assemble(x, results):
    """Full [B, 3L, V] output from per-core y tiles + host one-hot rows."""
    x = np.asarray(x, dtype=np.int64)
    y4 = np.zeros((B, S, L, V), np.float32)
    firsts = x[:, (np.arange(S) + 1) * L]               # [B, S]
    bi = np.repeat(np.arange(B), S)
    si = np.tile(np.arange(S), B)
    y4[bi, si, 0, firsts.reshape(-1)] = 1.0
    for j in range(NCORE):
        yj = results[j]["y"].astype(np.float32)
        yj = yj.reshape(LSLOT, S, B, -1)                # row = ls*G + s*B + b
        vs = yj.shape[-1]
        for ls, l in enumerate(LMAP[j]):
            if j == NCORE - 1 and ls == LSLOT - 1:
                continue  # dummy slot
            y4[:, :, l, :vs] = yj[ls].transpose(1, 0, 2)
    return y4.reshape(B, S * L, V)


def run(inputs: dict, nv_chunks: int = NCHUNK, trace: bool = False):
    nc = get_module(nv_chunks)
    in_maps = make_in_maps(
        inputs["x"], inputs["embedding"], inputs["W_csm"],
        inputs["Wx1"], inputs["Wh1"], inputs["Wx2"], inputs["Wh2"],
        inputs["U"], inputs["Ww"], inputs["Wc"], inputs["Wfc"])
    res = run_bass_kernel_spmd(
        nc, in_maps, core_ids=list(range(NCORE)), trace=trace)
    out = assemble(inputs["x"], res.results)
    return out, res


def kernel(**inputs) -> np.ndarray:
    out, _ = run(inputs)
    return out
